# revision 2
# baseline (speedup 1.0000x reference)
# Self-contained Trainium2 Bass kernel for nn_CoEvolutionaryJumpDiffusion.
#
# Strategy: data-parallel over the 1024 graph nodes -> 128 rows per core on 8
# NeuronCores. Each step every core computes z_next for its rows; one
# AllGather per step redistributes z in BOTH layouts ([rows,D] natural and
# [D,rows] transposed) so no per-step PE transposes of the gathered state are
# needed. All JAX PRNG draws depend only on the step index, so they are
# precomputed host-side (bit-exact threefry on CPU) and streamed to the device.
#
# On-device layout notes:
#  - Activations are kept transposed ([feature, node]) so every weight matrix
#    is a natural-layout stationary operand and biases are per-partition.
#  - softmax is computed without max-subtraction (logits are O(1) for this
#    model); denominator comes for free from a ones-column in the nb matmul.
#  - Only one ACT table set (natural_log_exp_and_others) is used: silu and
#    softplus are built from exp/ln + DVE ops to avoid ~2.7us table reloads.

import os
import subprocess
import sys
import tempfile

import numpy as np

N, D = 1024, 128
P = 128
NCORES = 8
ROWS = N // NCORES  # 128
DT_F = np.float32(0.05)
SQRT_DT = np.float32(0.05**0.5)
JUMP_MAG = np.float32(0.5)
STEPS = int(os.environ.get("BASS_KERNEL_STEPS", "200"))

_PRNG_SCRIPT = r"""
import sys, numpy as np
import jax, jax.numpy as jnp

steps = int(sys.argv[1])
out_path = sys.argv[2]
N, D = 1024, 128
DT = 0.05

base_key = jax.random.key(42)
dWs, us, jns = [], [], []
for i in range(steps):
    ki = jax.random.fold_in(base_key, i)
    kW, kU, kJ = jax.random.split(ki, 3)
    dWs.append(np.asarray(jax.random.normal(kW, (N, D), jnp.float32) * (DT**0.5)))
    us.append(np.asarray(jax.random.uniform(kU, (N,), jnp.float32)))
    jns.append(np.asarray(jax.random.normal(kJ, (N, D), jnp.float32) * 0.5))
np.savez(out_path, dW=np.stack(dWs), u=np.stack(us), jn=np.stack(jns))
"""

_prng_cache = {}


def _cpu_jax_env():
    """Env for a subprocess running plain CPU jax (no axon PJRT boot)."""
    import jax  # noqa: F401  (ensures jax is importable from this process)

    sp = os.path.dirname(os.path.dirname(jax.__file__))
    env = dict(os.environ)
    env.pop("TRN_TERMINAL_POOL_IPS", None)  # gates the axon sitecustomize boot
    env["JAX_PLATFORMS"] = "cpu"
    env["PYTHONPATH"] = sp
    return env


def _gen_randoms(steps):
    if steps in _prng_cache:
        return _prng_cache[steps]
    env = _cpu_jax_env()
    with tempfile.TemporaryDirectory() as td:
        out = os.path.join(td, "rng.npz")
        subprocess.run(
            [sys.executable, "-c", _PRNG_SCRIPT, str(steps), out],
            env=env,
            check=True,
            capture_output=True,
        )
        z = np.load(out)
        res = (z["dW"], z["u"], z["jn"])
    _prng_cache[steps] = res
    return res


_program_cache = {}


def _build_program(steps):
    """Build the SPMD Bass/Tile program (same NEFF for all 8 cores)."""
    import concourse.bass as bass
    import concourse.mybir as mybir
    import concourse.tile as tile
    from concourse import bacc
    from concourse.masks import make_identity

    dt = mybir.dt
    f32 = dt.float32
    AF = mybir.ActivationFunctionType
    ALU = mybir.AluOpType

    nc = bacc.Bacc(
        "TRN2",
        target_bir_lowering=False,
        debug=False,
        enable_asserts=False,
        num_devices=NCORES,
    )

    def inp(name, shape):
        return nc.dram_tensor(name, list(shape), f32, kind="ExternalInput").ap()

    # replicated weights (pre-processed host side)
    wWa = inp("wWa", [D, D])
    wd1a = inp("wd1a", [D, D])
    wd1b = inp("wd1b", [D, D])
    wd2s = inp("wd2s", [D, D])  # dW2 * DT
    ws1a = inp("ws1a", [D, D])
    ws2 = inp("ws2", [D, D])
    wi1a = inp("wi1a", [D, D])
    wi2 = inp("wi2", [D, 1])
    bd_pos = inp("bd_pos", [D, steps])
    bd_neg = inp("bd_neg", [D, steps])
    bs_pos = inp("bs_pos", [D, steps])
    bs_neg = inp("bs_neg", [D, steps])
    bi_pos = inp("bi_pos", [D, steps])
    bi_neg = inp("bi_neg", [D, steps])
    db2s = inp("db2s", [D, 1])  # db2 * DT
    sb2c = inp("sb2c", [D, 1])
    ib2c = inp("ib2c", [P, 1])  # ib2 broadcast over node partition
    # initial state (full, replicated) + per-core slice
    z0nat = inp("z0nat", [N, D])
    z0T = inp("z0T", [D, N])
    z0Tloc = inp("z0Tloc", [D, P])
    # per-core randoms
    u_all = inp("u_all", [P, steps])
    noise = inp("noise", [steps, P, D])  # 0.5 * normal, natural layout
    dWT = inp("dWT", [steps, D, P])  # sqrt(DT) * normal, transposed

    traj = nc.dram_tensor("traj", [steps, P, D], f32, kind="ExternalOutput").ap()
    jumps = nc.dram_tensor("jumps", [P, steps], f32, kind="ExternalOutput").ap()

    rg = [list(range(NCORES))]

    with tile.TileContext(nc, num_cores=NCORES) as tc:
        with (
            tc.tile_pool(name="persist", bufs=1) as PERS,
            tc.tile_pool(name="sb", bufs=2) as SB,
            tc.tile_pool(name="ps", bufs=1, space="PSUM") as PS,
            tc.tile_pool(name="dram", bufs=2, space="DRAM") as DRAM,
        ):
            ident = PERS.tile([P, P], f32, name="ident")
            make_identity(nc, ident)
            gath = [PERS.tile([P, 257], f32, name=f"gath{c}") for c in range(NCORES)]
            for c in range(NCORES):
                nc.gpsimd.memset(gath[c][:, 0:1], 1.0)
            zT_loc = PERS.tile([D, P], f32, name="zT_loc")
            jumps_acc = PERS.tile([P, steps], f32, name="jumps_acc")

            def load(name, ap_, shape):
                t = PERS.tile(list(shape), f32, name=f"w_{name}")
                nc.sync.dma_start(t[:], ap_)
                return t

            s_wWa = load("wWa", wWa, [D, D])
            s_wd1a = load("wd1a", wd1a, [D, D])
            s_wd1b = load("wd1b", wd1b, [D, D])
            s_wd2s = load("wd2s", wd2s, [D, D])
            s_ws1a = load("ws1a", ws1a, [D, D])
            s_ws2 = load("ws2", ws2, [D, D])
            s_wi1a = load("wi1a", wi1a, [D, D])
            s_wi2 = load("wi2", wi2, [D, 1])
            s_bdp = load("bdp", bd_pos, [D, steps])
            s_bdn = load("bdn", bd_neg, [D, steps])
            s_bsp = load("bsp", bs_pos, [D, steps])
            s_bsn = load("bsn", bs_neg, [D, steps])
            s_bip = load("bip", bi_pos, [D, steps])
            s_bin = load("bin", bi_neg, [D, steps])
            s_db2 = load("db2", db2s, [D, 1])
            s_sb2 = load("sb2", sb2c, [D, 1])
            s_ib2 = load("ib2", ib2c, [P, 1])
            s_u = load("u", u_all, [P, steps])

            # initial gathered state from z0 (full copies on every core)
            for c in range(NCORES):
                nc.sync.dma_start(
                    gath[c][:, 1:129], z0nat[c * P : (c + 1) * P, :]
                )
                nc.sync.dma_start(
                    gath[c][:, 129:257], z0T[:, c * P : (c + 1) * P]
                )
            nc.sync.dma_start(zT_loc[:], z0Tloc)

            def silu_from(ps_pre, bneg_ap, bpos_ap, tagbase):
                """silu(ps_pre + bias) computed with exp table only."""
                e = SB.tile([D, P], f32, tag=f"{tagbase}_e", name=f"{tagbase}_e")
                nc.scalar.activation(e[:], ps_pre[:], AF.Exp, bias=bneg_ap, scale=-1.0)
                t1 = SB.tile([D, P], f32, tag=f"{tagbase}_t", name=f"{tagbase}_t")
                nc.gpsimd.tensor_scalar_add(t1[:], e[:], 1.0)
                r = SB.tile([D, P], f32, tag=f"{tagbase}_r", name=f"{tagbase}_r")
                nc.vector.reciprocal_approx_fast(r[:], t1[:])
                s = SB.tile([D, P], f32, tag=f"{tagbase}_s", name=f"{tagbase}_s")
                nc.vector.scalar_tensor_tensor(
                    out=s[:],
                    in0=ps_pre[:],
                    scalar=bpos_ap,
                    in1=r[:],
                    op0=ALU.add,
                    op1=ALU.mult,
                )
                return s

            for s in range(steps):
                sc = slice(s, s + 1)
                # ---------- local phase (independent of the AllGather) ----
                ps_q = PS.tile([D, P], f32, tag="q", bufs=2, name="ps_q")
                nc.tensor.matmul(ps_q[:], s_wWa[:], zT_loc[:])
                qT = SB.tile([D, P], f32, tag="qT", name="qT")
                nc.scalar.copy(qT[:], ps_q[:])

                # diffusion MLP (local): silu(sW1a.T @ zT + bias) -> sW2 -> softplus
                ps_m1 = PS.tile([D, P], f32, tag="mlp", bufs=2, name="ps_m1")
                nc.tensor.matmul(ps_m1[:], s_ws1a[:], zT_loc[:])
                s1s = silu_from(ps_m1, s_bsn[:, sc], s_bsp[:, sc], "sl_s")
                ps_m2 = PS.tile([D, P], f32, tag="mlp", bufs=2, name="ps_m2")
                nc.tensor.matmul(ps_m2[:], s_ws2[:], s1s[:])
                e_s2 = SB.tile([D, P], f32, tag="e_s2", name="e_s2")
                nc.scalar.activation(e_s2[:], ps_m2[:], AF.Exp, bias=s_sb2[:, 0:1])
                t_s2 = SB.tile([D, P], f32, tag="t_s2", name="t_s2")
                nc.gpsimd.tensor_scalar_add(t_s2[:], e_s2[:], 1.0)
                diffT = SB.tile([D, P], f32, tag="diffT", name="diffT")
                nc.scalar.activation(diffT[:], t_s2[:], AF.Ln)

                # intensity MLP (local) -> lam -> jump mask
                ps_m3 = PS.tile([D, P], f32, tag="mlp", bufs=2, name="ps_m3")
                nc.tensor.matmul(ps_m3[:], s_wi1a[:], zT_loc[:])
                s2i = silu_from(ps_m3, s_bin[:, sc], s_bip[:, sc], "sl_i")
                ps_lam = PS.tile([P, 1], f32, tag="q", bufs=2, name="ps_lam")
                nc.tensor.matmul(ps_lam[:], s2i[:], s_wi2[:])
                e_lam = SB.tile([P, 1], f32, tag="e_lam", name="e_lam")
                nc.scalar.activation(e_lam[:], ps_lam[:], AF.Exp, bias=s_ib2[:, 0:1])
                t_lam = SB.tile([P, 1], f32, tag="t_lam", name="t_lam")
                nc.gpsimd.tensor_scalar_add(t_lam[:], e_lam[:], 1.0)
                lam = SB.tile([P, 1], f32, tag="lam", name="lam")
                nc.scalar.activation(lam[:], t_lam[:], AF.Ln)
                lamdt = SB.tile([P, 1], f32, tag="lamdt", name="lamdt")
                nc.vector.tensor_scalar_mul(lamdt[:], lam[:], float(DT_F))
                nc.vector.tensor_tensor(
                    jumps_acc[:, sc], s_u[:, sc], lamdt[:], ALU.is_lt
                )

                # per-step noise loads
                jn = SB.tile([P, D], f32, tag="jn", bufs=3, name="jn")
                nc.sync.dma_start(jn[:], noise[s])
                dwt = SB.tile([D, P], f32, tag="dwt", bufs=3, name="dwt")
                nc.sync.dma_start(dwt[:], dWT[s])
                t2 = SB.tile([D, P], f32, tag="t2", name="t2")
                nc.vector.tensor_tensor(t2[:], diffT[:], dwt[:], ALU.mult)

                # ---------- attention (needs gathered z of this step) -----
                ps_lt = [
                    PS.tile([P, 512], f32, tag="lt", bufs=2, name=f"ps_lt{g}")
                    for g in range(2)
                ]
                for g in range(2):
                    for cc in range(4):
                        c = 4 * g + cc
                        nc.tensor.matmul(
                            ps_lt[g][:, cc * P : (cc + 1) * P],
                            gath[c][:, 129:257],
                            qT[:],
                        )
                atg = [
                    SB.tile([P, 512], f32, tag="at", bufs=2, name=f"atg{g}")
                    for g in range(2)
                ]
                ps_nb = PS.tile([P, 129], f32, tag="nb", bufs=1, name="ps_nb")
                for g in range(2):
                    nc.scalar.activation(atg[g][:], ps_lt[g][:], AF.Exp)
                    for cc in range(4):
                        c = 4 * g + cc
                        nc.tensor.matmul(
                            ps_nb[:],
                            atg[g][:, cc * P : (cc + 1) * P],
                            gath[c][:, 0:129],
                            start=(c == 0),
                            stop=(c == NCORES - 1),
                        )

                rs = SB.tile([P, 1], f32, tag="rs", name="rs")
                nc.vector.reciprocal(rs[:], ps_nb[:, 0:1])
                nbn = SB.tile([P, D], f32, tag="nbn", name="nbn")
                nc.vector.tensor_scalar_mul(nbn[:], ps_nb[:, 1:129], rs[:])
                ps_tn = PS.tile([D, P], f32, tag="tp", bufs=1, name="ps_tn")
                nc.tensor.transpose(ps_tn[:], nbn[:], ident[:])
                nbT = SB.tile([D, P], f32, tag="nbT", name="nbT")
                nc.scalar.copy(nbT[:], ps_tn[:])

                # drift MLP
                ps_m4 = PS.tile([D, P], f32, tag="mlp", bufs=2, name="ps_m4")
                nc.tensor.matmul(ps_m4[:], s_wd1a[:], zT_loc[:], start=True, stop=False)
                nc.tensor.matmul(ps_m4[:], s_wd1b[:], nbT[:], start=False, stop=True)
                s1d = silu_from(ps_m4, s_bdn[:, sc], s_bdp[:, sc], "sl_d")
                ps_m5 = PS.tile([D, P], f32, tag="mlp", bufs=2, name="ps_m5")
                nc.tensor.matmul(ps_m5[:], s_wd2s[:], s1d[:], start=True, stop=False)
                nc.tensor.matmul(ps_m5[:], ident[:], zT_loc[:], start=False, stop=True)

                # z_pre (transposed) = (drift*DT + db2*DT + z) + diff .* dW
                zpre = SB.tile([D, P], f32, tag="zpre", name="zpre")
                nc.vector.scalar_tensor_tensor(
                    out=zpre[:],
                    in0=ps_m5[:],
                    scalar=s_db2[:, 0:1],
                    in1=t2[:],
                    op0=ALU.add,
                    op1=ALU.add,
                )
                ps_t1 = PS.tile([P, D], f32, tag="tp", bufs=1, name="ps_t1")
                nc.tensor.transpose(ps_t1[:], zpre[:], ident[:])
                znn = SB.tile([P, D], f32, tag="znn", bufs=2, name="znn")
                nc.vector.scalar_tensor_tensor(
                    out=znn[:],
                    in0=jn[:],
                    scalar=jumps_acc[:, sc],
                    in1=ps_t1[:],
                    op0=ALU.mult,
                    op1=ALU.add,
                )
                nc.sync.dma_start(traj[s], znn[:])
                ps_t2 = PS.tile([D, P], f32, tag="tp", bufs=1, name="ps_t2")
                nc.tensor.transpose(ps_t2[:], znn[:], ident[:])
                nc.scalar.copy(zT_loc[:], ps_t2[:])

                if s < steps - 1:
                    agin = DRAM.tile([P, 256], f32, tag="agin", name="agin")
                    agout = DRAM.tile(
                        [N, 256], f32, tag="agout", name="agout", addr_space="Shared"
                    )
                    nc.sync.dma_start(agin[:, 0:128], znn[:])
                    nc.sync.dma_start(agin[:, 128:256], zT_loc[:])
                    nc.gpsimd.collective_compute(
                        "AllGather",
                        mybir.AluOpType.bypass,
                        replica_groups=rg,
                        ins=[agin.opt()],
                        outs=[agout.opt()],
                    )
                    for c in range(NCORES):
                        nc.sync.dma_start(
                            gath[c][:, 1:257], agout[c * P : (c + 1) * P, :]
                        )

            nc.sync.dma_start(jumps, jumps_acc[:])

    nc.compile()
    return nc


def _get_program(steps):
    if steps not in _program_cache:
        _program_cache[steps] = _build_program(steps)
    return _program_cache[steps]


def _prepare_in_maps(inputs, steps):
    f32 = np.float32
    g = {k: np.asarray(v, dtype=f32) for k, v in inputs.items()}
    z0 = g["z0"]
    dW1, db1 = g["dW1"], g["db1"]
    sW1, sb1 = g["sW1"], g["sb1"]
    iW1, ib1 = g["iW1"], g["ib1"]

    t = (np.arange(steps, dtype=f32) * DT_F).astype(f32)  # [S]
    # fused layer-1 biases: db + t * w_last  -> [D, S]
    bd = (db1[None, :] + t[:, None] * dW1[2 * D][None, :]).astype(f32).T.copy()
    bs = (sb1[None, :] + t[:, None] * sW1[D][None, :]).astype(f32).T.copy()
    bi = (ib1[None, :] + t[:, None] * iW1[D][None, :]).astype(f32).T.copy()

    dW_all, u_all, jn_all = _gen_randoms(steps)

    common = {
        "wWa": np.ascontiguousarray(g["W_a"]),
        "wd1a": np.ascontiguousarray(dW1[0:D]),
        "wd1b": np.ascontiguousarray(dW1[D : 2 * D]),
        "wd2s": np.ascontiguousarray(g["dW2"] * DT_F),
        "ws1a": np.ascontiguousarray(sW1[0:D]),
        "ws2": np.ascontiguousarray(g["sW2"]),
        "wi1a": np.ascontiguousarray(iW1[0:D]),
        "wi2": np.ascontiguousarray(g["iW2"]),
        "bd_pos": bd,
        "bd_neg": (-bd).astype(f32),
        "bs_pos": bs,
        "bs_neg": (-bs).astype(f32),
        "bi_pos": bi,
        "bi_neg": (-bi).astype(f32),
        "db2s": (g["db2"] * DT_F).reshape(D, 1),
        "sb2c": g["sb2"].reshape(D, 1),
        "ib2c": np.full((P, 1), g["ib2"][0], dtype=f32),
        "z0nat": np.ascontiguousarray(z0),
        "z0T": np.ascontiguousarray(z0.T),
    }

    in_maps = []
    for k in range(NCORES):
        rows = slice(k * ROWS, (k + 1) * ROWS)
        m = dict(common)
        m["z0Tloc"] = np.ascontiguousarray(z0.T[:, rows])
        m["u_all"] = np.ascontiguousarray(u_all[:, rows].T)
        m["noise"] = np.ascontiguousarray(jn_all[:, rows, :])
        m["dWT"] = np.ascontiguousarray(np.swapaxes(dW_all[:, rows, :], 1, 2))
        in_maps.append(m)
    return in_maps, z0


_last_results = {}


def kernel(**inputs):
    from concourse.bass_utils import run_bass_kernel_spmd

    steps = STEPS
    nc = _get_program(steps)
    in_maps, z0 = _prepare_in_maps(inputs, steps)

    trace = os.environ.get("BASS_KERNEL_TRACE", "0") == "1"
    res = run_bass_kernel_spmd(
        nc,
        in_maps,
        core_ids=list(range(NCORES)),
        trace=trace,
    )
    _last_results["res"] = res

    trajectory = np.empty((steps + 1, N, D), dtype=np.float32)
    trajectory[0] = z0
    jumps = np.empty((steps, N), dtype=np.float32)
    for k in range(NCORES):
        rows = slice(k * ROWS, (k + 1) * ROWS)
        out = res.results[k]
        trajectory[1:, rows, :] = out["traj"]
        jumps[:, rows] = out["jumps"].T
    return trajectory, jumps
